# revision 33
# baseline (speedup 1.0000x reference)
"""Trainium2 Bass kernel for nn_FactorCovModel.

Model: 2-layer LSTM (H=512) over [B=256, T=64, D=500], last hidden ->
FC [512 -> 16532] -> Sigma = Lambda diag(exp(fv)) Lambda^T + diag(exp(idio)),
output [256, 500, 500].

Sharding: pure data parallel over batch, 32 samples/core on 8 cores.

Per-core device algorithm (matmul operands bf16, fp32 PSUM accumulation):
  - Weight gate axis host-permuted so PSUM col group hg holds hidden slice
    hg of ALL FOUR gates ordered [i, f, o, g]: PSUM [128 = (hg, batch),
    512 = i|f|o|g x 128].  Sigmoid covers cols 0:384, tanh 384:512, so the
    gate nonlinearity is 2 ACT ops.
  - LSTM gates are column-tiled: stationary = hT chunk [128, 32], 4
    hidden-slice groups run concurrently at tile positions (0, 32j).
  - Layer 1 runs ONE TIMESTEP BEHIND layer 0 (software pipeline): per
    period the PE issue order is [xproj(p+1) | trh0T(p) | rec(p+1) |
    trh1T(p-2) | L1 bias+inproj(p-1) | L1 rec(p-1)], so PE always has
    dependency-free work while ACT/DVE run the gate nonlinearities.
  - L1 bias is injected via a K=1 matmul (ones stationary x bias row), so
    PSUM evacuation is plain copies split across ACT and DVE.
  - PSUM: 4 banks hold L0 gates; the other 4 rotate trh0T -> trh1T -> L1
    gates within each period.
  - FC runs col-packed (4 feature tiles of 512 per PSUM tile), then Lambda
    is re-laid-out via 500 PE transposes of [32, 32] blocks into
    LT [32 factors, 500 assets, 32 batch]; fvar gets exp via ACT.
  - Sigma_b = (LT_b * f_b)^T @ LT_b per sample, 4 m-tiles of 128.
  - idio raw rows go back to the host, which applies bias+exp and adds the
    diagonal (avoids diagonal APs on device).
"""

import os
import sys

sys.path.insert(0, "/opt/trn_rl_repo")

import numpy as np

import concourse.bass as bass
import concourse.mybir as mybir
from concourse import bacc
from concourse.tile import TileContext

FP = mybir.dt.float32
BF = mybir.dt.bfloat16
AF = mybir.ActivationFunctionType

B_FULL, T_FULL, D_IN, H = 256, 64, 500, 512
NCORES = 8
BL = B_FULL // NCORES            # 32 samples per core
NA, NF = 500, 32                 # assets, factors
OUT_DIM = NA * NF + NF + NA      # 16532
NTILE = 512                      # FC feature tile
N_FTILES = 33                    # ceil(16532/512) -> features padded to 16896
FH = N_FTILES * NTILE            # 16896
XCHUNK = 16                      # time steps per streamed xT chunk

# gate-axis permutation: new col (hg, pos, hl) = 512*hg + 128*pos + hl maps to
# old row gate*512 + 128*hg + hl with gate order [i, f, o, g] (torch order is
# [i, f, g, o]).  With this layout PSUM col group hg holds ALL FOUR gates of
# hidden slice hg along the free dim, sigmoid spans cols 0:384 and tanh
# 384:512, so the nonlinearity is 2 full-partition ACT ops.
GSEL = [0, 1, 3, 2]              # new pos -> torch gate index
PERM = np.array([GSEL[pos] * 512 + 128 * hg + hl
                 for hg in range(4) for pos in range(4) for hl in range(128)])


# ---------------------------------------------------------------- host prep

def host_prep_shared(inputs):
    w_ih0 = np.asarray(inputs["w_ih0"])[PERM]
    w_hh0 = np.asarray(inputs["w_hh0"])[PERM]
    b0 = (np.asarray(inputs["b_ih0"]) + np.asarray(inputs["b_hh0"]))[PERM]
    w_ih1 = np.asarray(inputs["w_ih1"])[PERM]
    w_hh1 = np.asarray(inputs["w_hh1"])[PERM]
    b1 = (np.asarray(inputs["b_ih1"]) + np.asarray(inputs["b_hh1"]))[PERM]
    fc_w = np.asarray(inputs["fc_w"])
    fc_b = np.asarray(inputs["fc_b"])

    w0T = np.zeros((512, 2048), np.float32)
    w0T[:500] = w_ih0.T
    w0T[500] = b0
    wh0T = np.ascontiguousarray(w_hh0.T, dtype=np.float32)
    wi1T = np.ascontiguousarray(w_ih1.T, dtype=np.float32)
    wh1T = np.ascontiguousarray(w_hh1.T, dtype=np.float32)
    b1row = np.ascontiguousarray(b1.reshape(1, 2048), dtype=np.float32)
    ones1 = np.ones((1, 32), np.float32)
    fcwT = np.zeros((512, FH), np.float32)
    fcwT[:, :OUT_DIM] = fc_w.T
    fcbRow = np.zeros((1, FH), np.float32)
    fcbRow[0, :OUT_DIM] = fc_b
    ident = np.ascontiguousarray(np.tile(np.eye(32, dtype=np.float32), (4, 1)))
    import ml_dtypes
    tobf = lambda a: np.ascontiguousarray(a, dtype=ml_dtypes.bfloat16)
    return dict(w0T=tobf(w0T), wh0T=tobf(wh0T), wi1T=tobf(wi1T),
                wh1T=tobf(wh1T), b1row=tobf(b1row), ones1=tobf(ones1),
                fcwT=tobf(fcwT), fcbRow=tobf(fcbRow), identt=ident)


def host_prep_x(x_core):
    """x_core [BL, T, 500] -> xT [512, T*BL], (t, b) free order, ones bias row."""
    T = x_core.shape[1]
    import ml_dtypes
    xT = np.zeros((512, T * BL), np.float32)
    xT[:500] = np.asarray(x_core, np.float32).transpose(2, 1, 0).reshape(500, T * BL)
    xT[500] = 1.0
    return np.ascontiguousarray(xT, dtype=ml_dtypes.bfloat16)


# ---------------------------------------------------------------- bass build

def build_nc(T=T_FULL):
    nc = bacc.Bacc("TRN2")

    xT_d = nc.dram_tensor("xT", [512, T * BL], BF, kind="ExternalInput")
    w0T_d = nc.dram_tensor("w0T", [512, 2048], BF, kind="ExternalInput")
    wh0T_d = nc.dram_tensor("wh0T", [512, 2048], BF, kind="ExternalInput")
    wi1T_d = nc.dram_tensor("wi1T", [512, 2048], BF, kind="ExternalInput")
    wh1T_d = nc.dram_tensor("wh1T", [512, 2048], BF, kind="ExternalInput")
    b1row_d = nc.dram_tensor("b1row", [1, 2048], BF, kind="ExternalInput")
    ones1_d = nc.dram_tensor("ones1", [1, 32], BF, kind="ExternalInput")
    fcwT_d = nc.dram_tensor("fcwT", [512, FH], BF, kind="ExternalInput")
    fcbRow_d = nc.dram_tensor("fcbRow", [1, FH], BF, kind="ExternalInput")
    identt_d = nc.dram_tensor("identt", [128, 32], FP, kind="ExternalInput")

    sigma_d = nc.dram_tensor("sigma", [BL, NA, NA], BF, kind="ExternalOutput")
    idio_d = nc.dram_tensor("idio_raw", [BL, NA], FP, kind="ExternalOutput")

    def mm(out, lhsT, rhs, tp, **kw):
        nc.tensor.matmul(out, lhsT, rhs,
                         tile_position=tp, skip_group_check=True, **kw)

    def tr(out, in_, identity, tp):
        nc.tensor.matmul(out, in_, identity, is_transpose=True,
                         tile_position=tp, skip_group_check=True)

    with TileContext(nc) as tc:
        with tc.tile_pool(name="persist", bufs=1) as persist:
            identt_sb = persist.tile([128, 32], FP)
            nc.sync.dma_start(identt_sb, identt_d[:, :])
            identtb_sb = persist.tile([128, 32], BF)
            nc.vector.tensor_copy(identtb_sb, identt_sb)
            fcbRow_sb = persist.tile([1, FH], BF)
            nc.sync.dma_start(fcbRow_sb, fcbRow_d[:, :])
            b1row_sb = persist.tile([1, 2048], BF)
            nc.sync.dma_start(b1row_sb, b1row_d[:, :])
            ones1_sb = persist.tile([1, 32], BF)
            nc.sync.dma_start(ones1_sb, ones1_d[:, :])
            hlast = persist.tile([128, 128], BF)  # final h1T, chunk-major cols
            # FC weight prefetch: first NPRE tiles stream in during the LSTM
            # phase (DMA engines are otherwise idle there).
            NPRE = 18
            fcw_pre = persist.tile([128, 4, NPRE * 512], BF)

            # ---------------- phase 1: LSTM ----------------
            with (
                tc.tile_pool(name="wconst", bufs=1) as wconst,
                tc.tile_pool(name="xring", bufs=2) as xring,
                tc.tile_pool(name="state", bufs=1) as state,
                tc.tile_pool(name="work", bufs=2) as work,
                tc.tile_pool(name="pg0", bufs=4, space="PSUM") as pg0,
                tc.tile_pool(name="pgs", bufs=4, space="PSUM") as pgs,
            ):
                w0T_sb = wconst.tile([128, 4, 2048], BF)
                nc.sync.dma_start(w0T_sb, w0T_d.rearrange("(ko p) g -> p ko g", p=128))
                wh0T_sb = wconst.tile([128, 4, 2048], BF)
                nc.sync.dma_start(wh0T_sb, wh0T_d.rearrange("(ko p) g -> p ko g", p=128))
                wi1T_sb = wconst.tile([128, 4, 2048], BF)
                nc.sync.dma_start(wi1T_sb, wi1T_d.rearrange("(ko p) g -> p ko g", p=128))
                wh1T_sb = wconst.tile([128, 4, 2048], BF)
                nc.sync.dma_start(wh1T_sb, wh1T_d.rearrange("(ko p) g -> p ko g", p=128))

                xch = min(XCHUNK, T)
                n_xchunks = (T + xch - 1) // xch
                x_tiles = {}

                def load_xchunk(ci):
                    if ci >= n_xchunks:
                        return
                    xt = xring.tile([128, 4, xch * BL], BF, tag="xchunk")
                    nc.sync.dma_start(
                        xt,
                        xT_d[:, ci * xch * BL:(ci + 1) * xch * BL]
                        .rearrange("(ko p) tb -> p ko tb", p=128),
                    )
                    x_tiles[ci] = xt

                load_xchunk(0)
                # prefetch in FC-consumption order: quad 7 (jj 28-31) first
                nc.sync.dma_start(
                    fcw_pre[:, :, 0:4 * 512],
                    fcwT_d[:, 28 * 512:32 * 512]
                    .rearrange("(ko p) n -> p ko n", p=128))
                nc.sync.dma_start(
                    fcw_pre[:, :, 4 * 512:NPRE * 512],
                    fcwT_d[:, 0:(NPRE - 4) * 512]
                    .rearrange("(ko p) n -> p ko n", p=128))

                def xproj(t, banks):
                    """L0 input-projection matmuls for step t (start of accum)."""
                    ci, tl = t // xch, t % xch
                    if tl == 0:
                        load_xchunk(ci + 1)
                    xt = x_tiles[ci]
                    last = (t == 0)  # no recurrent part at t == 0
                    for k in range(4):
                        lhsT = xt[:, k, tl * BL:(tl + 1) * BL]
                        for j in range(4):
                            mm(banks[j][32 * j:32 * (j + 1), :], lhsT,
                               w0T_sb[:, k, 512 * j:512 * (j + 1)],
                               tp=(0, 32 * j),
                               start=(k == 0), stop=(last and k == 3))

                def recur(hT, w_sb, banks, start, stop):
                    """4-chunk recurrent (or in-proj) matmuls accumulating."""
                    for k in range(4):
                        lhsT = hT[:, 32 * k:32 * (k + 1)]
                        for j in range(4):
                            mm(banks[j][32 * j:32 * (j + 1), :], lhsT,
                               w_sb[:, k, 512 * j:512 * (j + 1)],
                               tp=(0, 32 * j),
                               start=(start and k == 0), stop=(stop and k == 3))

                def bias_l1(banks):
                    """Inject b1 via K=1 matmul: ones [1,32] x b1row [1,512]."""
                    for j in range(4):
                        mm(banks[j][32 * j:32 * (j + 1), :], ones1_sb[0:1, :],
                           b1row_sb[0:1, 512 * j:512 * (j + 1)],
                           tp=(0, 32 * j), start=True, stop=False)

                def evac(banks, dst, bias=None):
                    """Bank j -> dst[32j:32j+32]: plain copies split ACT/DVE,
                    or +bias as 4 DVE adds (L1: latency has pipeline slack)."""
                    for j in range(4):
                        s = slice(32 * j, 32 * (j + 1))
                        if bias is not None:
                            nc.vector.tensor_add(dst[s, :], banks[j][s, :],
                                                 bias[s, :])
                        elif j % 2 == 0:
                            nc.scalar.copy(dst[s, :], banks[j][s, :])
                        else:
                            nc.vector.tensor_copy(dst[s, :], banks[j][s, :])
                    return dst

                def gate_nonlin(ga, cprev, cnew, tag, eng):
                    """ga [128=(hg,b), 512 = i|f|o|g x128] SBUF -> (hh, cnew).
                    Elementwise muls/adds run on `eng` (DVE for the
                    latency-critical L0 chain, GpSimd for the slack L1)."""
                    a = work.tile([128, 512], FP, tag=f"act_{tag}")
                    nc.scalar.activation(a[:, 0:384], ga[:, 0:384], AF.Sigmoid)
                    nc.scalar.activation(a[:, 384:512], ga[:, 384:512], AF.Tanh)
                    t1 = work.tile([128, 128], FP, tag=f"t1_{tag}")
                    if cprev is not None:
                        # t2 on the other engine, in parallel with t1
                        t2 = work.tile([128, 128], FP, tag=f"t2_{tag}")
                        eng2 = nc.gpsimd if eng is nc.vector else nc.vector
                        eng2.tensor_mul(t2, a[:, 128:256], cprev)
                    eng.tensor_mul(t1, a[:, 0:128], a[:, 384:512])
                    if cprev is None:
                        cn = t1  # c_prev == 0 at t == 0
                    else:
                        cn = cnew
                        eng.tensor_add(cn, t1, t2)
                    tcn = work.tile([128, 128], FP, tag=f"tc_{tag}")
                    nc.scalar.activation(tcn, cn, AF.Tanh)
                    hh = work.tile([128, 128], BF, tag=f"h_{tag}")
                    eng.tensor_mul(hh, a[:, 256:384], tcn)
                    return hh, cn

                def transpose_h(hh, ht, split=False):
                    """hh bf16 [128=(hg,b),128] -> ht bf16 [128,128] via the
                    DMA XBAR transpose (no PE/PSUM/DVE involvement).  With
                    split=True, 4 column-chunk DMAs so the consumer's first
                    k-chunk matmul can start as soon as chunk 0 lands."""
                    if split:
                        for k in range(4):
                            nc.sync.dma_start_transpose(
                                ht[:, 32 * k:32 * (k + 1)],
                                hh[32 * k:32 * (k + 1), :])
                    else:
                        nc.sync.dma_start_transpose(ht, hh)
                    return ht

                # state ring buffers (explicit tags with own bufs counts)
                def h0T_tile(p):
                    return state.tile([128, 128], BF, tag="h0T", bufs=3,
                                      name=f"h0T_{p}")

                def h1T_tile(p):
                    return state.tile([128, 128], BF, tag="h1T", bufs=2,
                                      name=f"h1T_{p}")

                # pipeline registers
                h0T = {}          # p -> tile
                h1T = {}
                hh0 = {}          # raw h (pre-transpose) per step
                hh1 = {}
                c0 = c1 = None
                pgs0_banks = None  # live L0 gate banks (gen p+1)
                pgs1_banks = None

                def l0_nonlin(t):
                    nonlocal c0
                    a0 = work.tile([128, 512], FP, tag="ga_l0")
                    evac(pgs0_banks, a0)
                    c0n = None if c0 is None else state.tile(
                        [128, 128], FP, tag="c0", bufs=2, name=f"c0_{t}")
                    hh, c0 = gate_nonlin(a0, c0, c0n, "l0", nc.vector)
                    hh0[t] = hh

                def l1_nonlin(t):
                    nonlocal c1
                    a1 = work.tile([128, 512], FP, tag="ga_l1")
                    evac(pgs1_banks, a1)
                    c1n = None if c1 is None else state.tile(
                        [128, 128], FP, tag="c1", bufs=2, name=f"c1_{t}")
                    hh, c1 = gate_nonlin(a1, c1, c1n, "l1", nc.gpsimd)
                    hh1[t] = hh

                # prologue: xproj(0) complete + nonlin
                pgs0_banks = [pg0.tile([128, 512], FP, tag="g0", name=f"g0p{j}")
                              for j in range(4)]
                xproj(0, pgs0_banks)
                l0_nonlin(0)

                for p in range(T + 1):
                    # E: xproj(p+1) -> fresh L0 banks (start of accumulation)
                    if p + 1 <= T - 1:
                        pgs0_banks = [pg0.tile([128, 512], FP, tag="g0",
                                               name=f"g0_{p}_{j}")
                                      for j in range(4)]
                        xproj(p + 1, pgs0_banks)
                    # C: transpose h0(p)
                    if p <= T - 1:
                        h0T[p] = transpose_h(hh0.pop(p), h0T_tile(p))
                    # F: rec(p+1) closes L0 accumulation; then its nonlin
                    if p + 1 <= T - 1:
                        recur(h0T[p], wh0T_sb, pgs0_banks, start=False, stop=True)
                        l0_nonlin(p + 1)
                    # A: transpose h1(p-2)
                    if 2 <= p <= T + 1 and (p - 2) in hh1:
                        h1T[p - 2] = transpose_h(hh1.pop(p - 2), h1T_tile(p - 2))
                    # D: L1 bias + in-proj for step p-1
                    if 1 <= p <= T:
                        tl1 = p - 1
                        pgs1_banks = [pgs.tile([128, 512], FP, tag="gs",
                                               name=f"g1_{p}_{j}")
                                      for j in range(4)]
                        bias_l1(pgs1_banks)
                        recur(h0T.pop(tl1), wi1T_sb, pgs1_banks,
                              start=False, stop=(tl1 == 0))
                        # B: L1 recurrent for step p-1 (uses h1T(p-2))
                        if tl1 >= 1:
                            recur(h1T[tl1 - 1], wh1T_sb, pgs1_banks,
                                  start=False, stop=True)
                        l1_nonlin(tl1)

                # epilogue: transpose h1(T-1) into hlast (PE: low latency)
                transpose_h0_pe(hh1.pop(T - 1), hlast)

            # ---------------- phase 2: FC + Lambda layout + Sigma ----------------
            with tc.tile_pool(name="lt", bufs=1) as ltp:
                # LT holds [factor, asset, b] on partitions 0:32; LSQ4 is
                # sqrt(f)-scaled with 4 partition-replicas so sample 4g+r can
                # run its Sigma matmul on PE row tile r:
                # Sigma = (L sqrt(f)) (L sqrt(f))^T needs only ONE tensor.
                LT4 = ltp.tile([128, 500, 32], BF)
                LSQ4 = ltp.tile([128, 500, 32], BF)
                F_sb = ltp.tile([32, 32], FP)          # sqrt(exp(fvar+bias))

                with (
                    tc.tile_pool(name="fcw", bufs=3) as fcwp,
                    tc.tile_pool(name="rawp", bufs=3) as rawp,
                    tc.tile_pool(name="pfc", bufs=4, space="PSUM") as pfcp,
                    tc.tile_pool(name="plt", bufs=4, space="PSUM") as pltp,
                ):
                    def scale_and_replicate(a0, na):
                        """LSQ4[0:32] = LT * sqrt(f), replicated to partition
                        offsets 32/64/96 for the 4 PE row tiles."""
                        seng = nc.vector if (a0 // 64) % 2 == 0 else nc.gpsimd
                        seng.tensor_tensor(
                            LSQ4[0:32, a0:a0 + na, :], LT4[0:32, a0:a0 + na, :],
                            F_sb[:, None, :].to_broadcast([32, na, 32]),
                            mybir.AluOpType.mult,
                        )
                        for r in range(1, 4):
                            qeng = nc.sync if r == 2 else nc.scalar
                            qeng.dma_start(
                                LSQ4[32 * r:32 * (r + 1), a0:a0 + na, :],
                                LSQ4[0:32, a0:a0 + na, :])

                    # quad 7 first: it produces F_sb (fvar), unblocking the
                    # incremental scale+replicate for every quad.
                    for q in [7, 0, 1, 2, 3, 4, 5, 6, 8]:
                        rr = range(4) if q < 8 else range(1)
                        raw_t = rawp.tile([128, 512], FP, tag="raw")
                        for r in rr:
                            jj = 4 * q + r
                            if 28 <= jj <= 31:
                                pre = jj - 28
                            elif jj < NPRE - 4:
                                pre = jj + 4
                            else:
                                pre = None
                            if pre is not None:
                                fcw_t = fcw_pre[:, :, pre * 512:(pre + 1) * 512]
                            else:
                                fcw_t = fcwp.tile([128, 4, 512], BF, tag="fcw")
                                nc.sync.dma_start(
                                    fcw_t,
                                    fcwT_d[:, jj * 512:(jj + 1) * 512]
                                    .rearrange("(ko p) n -> p ko n", p=128),
                                )
                            # one PSUM bank per n-tile (col groups must not share)
                            pfc = pfcp.tile([128, 512], FP, tag="pfc")
                            for k in range(4):
                                mm(pfc[32 * r:32 * (r + 1), :],
                                   hlast[:, 32 * k:32 * (k + 1)],
                                   fcw_t[:, k, :],
                                   tp=(0, 32 * r),
                                   start=(k == 0), stop=False)
                            mm(pfc[32 * r:32 * (r + 1), :],
                               ones1_sb[0:1, :],
                               fcbRow_sb[0:1, jj * 512:(jj + 1) * 512],
                               tp=(0, 32 * r), start=False, stop=True)
                            s = slice(32 * r, 32 * (r + 1))
                            if r % 2 == 0:
                                nc.scalar.copy(raw_t[s, :], pfc[s, :])
                            else:
                                nc.vector.tensor_copy(raw_t[s, :], pfc[s, :])

                        # Lambda blocks: 16 sequential same-row-tile transposes
                        # into ONE psum bank (safe: same tile position), then a
                        # single 512-col DVE add into LT4.
                        for r in rr:
                            jj = 4 * q + r
                            nblk = 16 if jj < 31 else (4 if jj == 31 else 0)
                            if nblk:
                                pt = pltp.tile([32, 512], FP, tag="plt")
                                for blk in range(nblk):
                                    tr(pt[:, 32 * blk:32 * (blk + 1)],
                                       raw_t[32 * r:32 * (r + 1),
                                             32 * blk:32 * (blk + 1)],
                                       identt_sb[32 * r:32 * (r + 1), :],
                                       (32 * r, 0))
                                a0 = jj * 16           # first asset index
                                pt_v = (pt[:, 0:32 * nblk]
                                        .rearrange("f (a b) -> f a b", a=nblk))
                                if jj % 2 == 0:
                                    nc.scalar.copy(LT4[0:32, a0:a0 + nblk, :],
                                                   pt_v)
                                else:
                                    nc.vector.tensor_copy(
                                        LT4[0:32, a0:a0 + nblk, :], pt_v)
                            if jj == 31:
                                # fvar: features 16000:16032 = cols 128:160
                                ptf_full = pltp.tile([32, 512], FP, tag="plt")
                                ptf = ptf_full[:, 0:32]
                                tr(ptf, raw_t[96:128, 128:160],
                                   identt_sb[96:128, :], (96, 0))
                                nc.scalar.activation(F_sb, ptf, AF.Exp,
                                                     scale=0.5)
                                # idio part 1: features 16032:16384
                                nc.sync.dma_start(idio_d[:, 0:352],
                                                  raw_t[96:128, 160:512])
                            if jj == 32:
                                # idio part 2: features 16384:16532
                                nc.sync.dma_start(idio_d[:, 352:500],
                                                  raw_t[0:32, 0:148])
                        if q < 8:
                            a0 = 64 * q
                            scale_and_replicate(a0, min(64, NA - a0))

                # Sigma: scale once, replicate to 4 partition offsets, then
                # 4 samples run concurrently on the 4 PE row tiles.  Only the
                # block-upper-triangle (cols >= 128*mt) is computed + stored
                # in bf16; the host mirrors the rest (Sigma is symmetric).
                with (
                    tc.tile_pool(name="sigw", bufs=4) as sigw,
                    tc.tile_pool(name="psig", bufs=8, space="PSUM") as psigp,
                ):
                    for g in range(8):
                        for mt in range(4):
                            rows = 128 if mt < 3 else 116
                            ncols = 500 - 128 * mt
                            # one staging tile + ONE output DMA per 4 samples
                            st4 = sigw.tile([128, 4, 512], BF, tag="sigstage")
                            for r in range(4):
                                b = 4 * g + r
                                ps = psigp.tile([128, 512], FP, tag="psig")
                                mm(ps[:rows, 0:ncols],
                                   LSQ4[32 * r:32 * (r + 1),
                                        128 * mt:128 * mt + rows, b],
                                   LSQ4[32 * r:32 * (r + 1), 128 * mt:500, b],
                                   tp=(32 * r, 0), start=True, stop=True)
                                if r % 2 == 0:
                                    nc.scalar.copy(st4[:rows, r, 0:ncols],
                                                   ps[:rows, 0:ncols])
                                else:
                                    nc.vector.tensor_copy(st4[:rows, r, 0:ncols],
                                                          ps[:rows, 0:ncols])
                            qeng = nc.sync if (g * 4 + mt) % 2 == 0 else nc.scalar
                            qeng.dma_start(
                                sigma_d[4 * g:4 * (g + 1),
                                        128 * mt:128 * mt + rows, 128 * mt:500]
                                .rearrange("s r c -> r s c"),
                                st4[:rows, :, 0:ncols])

    nc.compile()
    return nc


# ---------------------------------------------------------------- entry point

def kernel(**inputs):
    from concourse.bass_utils import run_bass_kernel_spmd

    prep = host_prep_shared(inputs)
    x = np.asarray(inputs["x"], np.float32)
    in_maps = []
    for core in range(NCORES):
        m = dict(prep)
        m["xT"] = host_prep_x(x[core * BL:(core + 1) * BL])
        in_maps.append(m)

    nc = build_nc()
    res = run_bass_kernel_spmd(nc, in_maps, list(range(NCORES)))
    results = res.results

    idx = np.arange(NA)
    out = np.empty((B_FULL, NA, NA), np.float32)
    for core in range(NCORES):
        sigma = np.array(results[core]["sigma"], np.float32)
        # device stores only cols >= 128*(row//128); mirror the rest
        for mt in range(1, 4):
            cs = 128 * mt
            ce = min(cs + 128, NA)
            sigma[:, cs:ce, 0:cs] = np.swapaxes(sigma[:, 0:cs, cs:ce], 1, 2)
        idio = np.exp(np.asarray(results[core]["idio_raw"]))
        sigma[:, idx, idx] += idio.astype(np.float32)
        out[core * BL:(core + 1) * BL] = sigma
    return out
